# revision 28
# baseline (speedup 1.0000x reference)
"""EquivLayerNorm (segment_reduce) Trainium2 kernel.

Problem: per-graph layer-norm statistics over sorted contiguous segments.
  s [N, 128], v [N, 3, 128], index [N] sorted in [0, 512).
  counts  = segsum(1)
  smean   = segsum(mean_d(s)) / cnt
  var     = segsum(mean_d((s - smean)^2)) / cnt   == E[mean_d(s^2)] - smean^2
  sout    = (s - smean) / max(var, eps) * weight_s + bias_s
  vmean   = segsum(mean_d(sum_c v^2)) / cnt
  vout    = v / max(vmean, eps)

Strategy:
  * Host splits nodes at graph boundaries into 8 balanced shards (whole
    graphs per shard -> per-shard stats need no cross-core reduction).
  * Each core streams its nodes once in chunks of 2048. Per chunk it
    computes per-node partial sums (ScalarE activation accum), segment-
    reduces them into per-graph slots with a one-hot matmul, and keeps a
    running total. Because a graph spans at most 2 chunks (max graph 452
    nodes < 2048), chunk c can be normalized as soon as chunk c+1's stats
    are accumulated; per-graph params are gathered back to nodes with a
    transposed one-hot matmul. Single read of s/v from HBM, single write.
"""

import numpy as np

import concourse.bacc as bacc
import concourse.bass as bass  # noqa: F401 (AP helpers)
import concourse.tile as tile
from concourse import mybir
from concourse.bass_utils import run_bass_kernel_spmd

P = 128          # partitions
SD = 128         # s feature dim
VD = 384         # flattened v dim (3*128)
NUM_GRAPHS = 512
EPS = 1e-6
M_CORES = 8

# per-core streaming geometry (full problem)
R_FULL = 16            # 128-node rows per chunk
NCHUNK_FULL = 13       # chunks per core -> capacity 26624 >= max shard 25194
W_FULL = 64            # local graph-slot window per core (64 graphs per shard)

PAD_IDX = 200.0        # local index given to padding nodes; outside [0, W)
VN_ACT_ROWS = 4        # v-norm rows offloaded from DVE to ScalarE per chunk
VSQ_DVE_ROWS = 8       # v^2 rows done as batched ACT square + batched DVE reduce


def build_kernel(r=R_FULL, nchunk=NCHUNK_FULL, w=W_FULL, use_f32r=True,
                 store_eng='gpsimd'):
    """Builds the Bass kernel. Returns the traced nc."""
    ch = r * P
    cap = nchunk * ch
    f32 = mybir.dt.float32
    f32r = mybir.dt.float32r if use_f32r else mybir.dt.float32
    AF = mybir.ActivationFunctionType
    OP = mybir.AluOpType

    vsq_dve_rows = min(VSQ_DVE_ROWS, r)
    vn_act_rows = min(VN_ACT_ROWS, max(r - 1, 0))

    nc = bacc.Bacc("TRN2", target_bir_lowering=False)
    s_in = nc.dram_tensor("s_in", [cap, SD], f32, kind="ExternalInput").ap()
    v_in = nc.dram_tensor("v_in", [cap, VD], f32, kind="ExternalInput").ap()
    idx_in = nc.dram_tensor("idx_in", [cap], f32, kind="ExternalInput").ap()
    s_out = nc.dram_tensor("s_out", [cap, SD], f32, kind="ExternalOutput").ap()
    v_out = nc.dram_tensor("v_out", [cap, VD], f32, kind="ExternalOutput").ap()

    s_r = s_in.rearrange("(c r p) d -> c p r d", p=P, r=r)
    v_r = v_in.rearrange("(c r p) d -> c p r d", p=P, r=r)
    so_r = s_out.rearrange("(c r p) d -> c p r d", p=P, r=r)
    vo_r = v_out.rearrange("(c r p) d -> c p r d", p=P, r=r)
    idx_col_dram = idx_in.rearrange("(t p) -> p t", p=P)  # [P, nchunk*r]

    with tile.TileContext(nc) as tc, nc.allow_low_precision(
        reason="fp32r matmul operands hold pre-rounded partials"
    ):
        with (
            tc.tile_pool(name="consts", bufs=1) as consts,
            tc.tile_pool(name="sio", bufs=4) as sio,
            tc.tile_pool(name="vio", bufs=4) as vio,
            tc.tile_pool(name="idxb", bufs=4) as idxbp,
            tc.tile_pool(name="opool", bufs=2) as opool,
            tc.tile_pool(name="otpool", bufs=1) as otpool,
            tc.tile_pool(name="small", bufs=4) as small,
            tc.tile_pool(name="scr", bufs=2) as scr,
            tc.tile_pool(name="scrs2", bufs=1) as scrs2,
            tc.tile_pool(name="scrvb", bufs=1) as scrvb,
            tc.tile_pool(name="gath", bufs=3) as gathp,
            tc.tile_pool(name="ps_stats", bufs=2, space="PSUM") as ps_stats,
            tc.tile_pool(name="ps_gath", bufs=3, space="PSUM") as ps_gath,
        ):
            # constants
            iota_free_i = consts.tile([P, w], mybir.dt.int32)
            nc.gpsimd.iota(iota_free_i[:], pattern=[[1, w]], channel_multiplier=0)
            iota_free = consts.tile([P, w], f32)
            nc.vector.tensor_copy(iota_free[:], iota_free_i[:])

            iota_pc_i = consts.tile([P, 1], mybir.dt.int32)
            nc.gpsimd.iota(iota_pc_i[:], pattern=[[0, 1]], channel_multiplier=1)
            iota_pc = consts.tile([P, 1], f32)
            nc.vector.tensor_copy(iota_pc[:], iota_pc_i[:])

            # f32r constant of ones (is_equal(x, x) == 1.0); a plain memset
            # of an f32r tile fails the ISA check
            ones_row = consts.tile([P, r], f32r)
            nc.vector.tensor_tensor(
                out=ones_row[:], in0=iota_free[:, 0:r], in1=iota_free[:, 0:r],
                op=OP.is_equal,
            )

            idx_col = consts.tile([P, nchunk * r], f32)
            nc.sync.dma_start(out=idx_col[:], in_=idx_col_dram)

            held = {}

            # halves: batched ops are split in two so downstream engines
            # (PE matmuls, norms) start on the first half while the second
            # half is still being produced
            h2 = max(r // 2, 1)
            halves = [(0, h2), (h2, r)] if r > 1 else [(0, r)]

            def load_chunk(c):
                s_t = sio.tile([P, r, SD], f32, tag="s")
                v_t = vio.tile([P, r, VD], f32, tag="v")
                ib_t = idxbp.tile([P, ch], f32, tag="ib")
                nc.sync.dma_start(out=s_t[:], in_=s_r[c])
                nc.sync.dma_start(out=v_t[:], in_=v_r[c])
                nc.sync.dma_start(
                    out=ib_t[:],
                    in_=idx_in[c * ch : (c + 1) * ch][None, :].to_broadcast(
                        (P, ch)
                    ),
                )
                held[c] = (s_t, v_t, ib_t)

            def stats_chunk(c):
                """Segment-sum this chunk's per-node partials into [w, 4] PSUM.

                Per-row fixed instruction cost dominates at this size, so
                elementwise/reduce work runs as batched 3D-AP instructions
                (one per half-chunk). part is stat-major [P, 4, r] so each
                reduce writes a contiguous [P, r] slab.
                """
                s_t, v_t, _ = held[c]
                part = small.tile([P, 4, r], f32r, tag="part")
                nc.vector.tensor_copy(part[:, 3, :], ones_row[:])
                stats_ps = ps_stats.tile([w, 4], f32, tag="stats")
                vb = vsq_dve_rows
                for lo, hi in halves:
                    # batched per-node s sums
                    nc.vector.reduce_sum(
                        part[:, 0, lo:hi], s_t[:, lo:hi, :],
                        axis=mybir.AxisListType.X,
                    )
                    # batched s^2 on ScalarE, batched row-sums on DVE
                    scr_s2 = scrs2.tile([P, h2, SD], f32, tag="scr_s2")
                    nc.scalar.activation(
                        out=scr_s2[:, : hi - lo, :].rearrange("p r d -> p (r d)"),
                        in_=s_t[:, lo:hi, :].rearrange("p r d -> p (r d)"),
                        func=AF.Square,
                    )
                    nc.vector.reduce_sum(
                        part[:, 1, lo:hi], scr_s2[:, : hi - lo, :],
                        axis=mybir.AxisListType.X,
                    )
                    # v^2: batched (ACT square + DVE reduce) for rows < vb,
                    # per-row ScalarE free-dim accum for the rest
                    blo, bhi = min(lo, vb), min(hi, vb)
                    if bhi > blo:
                        scr_vb = scrvb.tile([P, h2, VD], f32, tag="scr_vb")
                        nc.scalar.activation(
                            out=scr_vb[:, : bhi - blo, :].rearrange(
                                "p r d -> p (r d)"
                            ),
                            in_=v_t[:, blo:bhi, :].rearrange("p r d -> p (r d)"),
                            func=AF.Square,
                        )
                        nc.vector.reduce_sum(
                            part[:, 2, blo:bhi], scr_vb[:, : bhi - blo, :],
                            axis=mybir.AxisListType.X,
                        )
                    for i in range(max(lo, vb), hi):
                        scr_v = scr.tile([P, VD], f32, tag="scr_v")
                        nc.scalar.activation(
                            out=scr_v[:], in_=v_t[:, i, :], func=AF.Square,
                            accum_out=part[:, 2, i : i + 1],
                        )
                    # batched one-hot for this half
                    o_t = opool.tile([P, h2, w], f32r, tag="o")
                    nc.vector.tensor_tensor(
                        out=o_t[:, : hi - lo, :],
                        in0=iota_free[:, None, :].to_broadcast((P, hi - lo, w)),
                        in1=idx_col[
                            :, c * r + lo : c * r + hi, None
                        ].to_broadcast((P, hi - lo, w)),
                        op=OP.is_equal,
                    )
                    for i in range(lo, hi):
                        nc.tensor.matmul(
                            stats_ps[:],
                            lhsT=o_t[:, i - lo, :], rhs=part[:, :, i],
                            start=(i == 0), stop=(i == r - 1),
                        )
                return stats_ps

            def params_from(totals):
                """totals [w, 4] sbuf -> G [w, 3] = (-smean, 1/var, 1/vmean).

                Only immediate-scalar tensor_scalar ops here: the AP-scalar
                variant (TensorScalarPtr) has too few instruction wait slots
                for Tile's sync and fails walrus codegen.
                """
                g_t = small.tile([w, 4], f32r, tag="G")
                t_sc = small.tile([w, 4], f32, tag="psc")
                rc = t_sc[:, 0:1]
                # rc = 1 / max(cnt, 1)
                nc.vector.tensor_scalar_max(out=rc, in0=totals[:, 3:4], scalar1=1.0)
                nc.vector.reciprocal(out=rc, in_=rc)
                # -m = -(S1 * rc) / SD
                nc.vector.tensor_tensor(
                    out=t_sc[:, 1:2], in0=totals[:, 0:1], in1=rc, op=OP.mult
                )
                nc.vector.tensor_scalar(
                    out=g_t[:, 0:1], in0=t_sc[:, 1:2], scalar1=-1.0 / SD,
                    scalar2=None, op0=OP.mult,
                )
                # Es2 = S2 * rc / SD
                nc.vector.tensor_tensor(
                    out=t_sc[:, 1:2], in0=totals[:, 1:2], in1=rc, op=OP.mult
                )
                nc.vector.tensor_scalar(
                    out=t_sc[:, 1:2], in0=t_sc[:, 1:2], scalar1=1.0 / SD,
                    scalar2=None, op0=OP.mult,
                )
                # var = max(Es2 - m^2, eps); a_s = 1/var   ((-m)^2 == m^2)
                nc.vector.tensor_tensor(
                    out=t_sc[:, 2:3], in0=g_t[:, 0:1], in1=g_t[:, 0:1], op=OP.mult
                )
                nc.vector.tensor_tensor(
                    out=t_sc[:, 1:2], in0=t_sc[:, 1:2], in1=t_sc[:, 2:3],
                    op=OP.subtract,
                )
                nc.vector.tensor_scalar_max(
                    out=t_sc[:, 1:2], in0=t_sc[:, 1:2], scalar1=EPS
                )
                nc.vector.reciprocal(out=g_t[:, 1:2], in_=t_sc[:, 1:2])
                # vmean = max(V * rc / SD, eps); a_v = 1/vmean
                nc.vector.tensor_tensor(
                    out=t_sc[:, 3:4], in0=totals[:, 2:3], in1=rc, op=OP.mult
                )
                nc.vector.tensor_scalar(
                    out=t_sc[:, 3:4], in0=t_sc[:, 3:4], scalar1=1.0 / SD,
                    scalar2=None, op0=OP.mult,
                )
                nc.vector.tensor_scalar_max(
                    out=t_sc[:, 3:4], in0=t_sc[:, 3:4], scalar1=EPS
                )
                nc.vector.reciprocal(out=g_t[:, 2:3], in_=t_sc[:, 3:4])
                # pad column so the fp32r gather matmul has an even free dim
                nc.vector.tensor_copy(g_t[:, 3:4], g_t[:, 2:3])
                return g_t

            def normalize_chunk(c, g_t):
                s_t, v_t, ib_t = held.pop(c)
                store = getattr(nc, store_eng)
                for lo, hi in halves:
                    n_h = hi - lo
                    # batched transposed one-hot for this half [P, n_h, 128]
                    ot_t = otpool.tile([P, h2, P], f32r, tag="ot")
                    nc.vector.tensor_tensor(
                        out=ot_t[:, :n_h, :],
                        in0=ib_t[:, lo * P : hi * P].rearrange(
                            "p (r n) -> p r n", r=n_h
                        ),
                        in1=iota_pc[:, :, None].to_broadcast((P, n_h, P)),
                        op=OP.is_equal,
                    )
                    # the half's row-gathers land in one PSUM tile -> one copy
                    g_ps = ps_gath.tile([P, h2, 4], f32, tag="gps")
                    for i in range(n_h):
                        nc.tensor.matmul(
                            g_ps[:, i, :],
                            lhsT=ot_t[:w, i, :], rhs=g_t[:w, :],
                            start=True, stop=True,
                        )
                    gth = gathp.tile([P, h2, 4], f32, tag="gth")
                    nc.vector.tensor_copy(gth[:, :n_h, :], g_ps[:, :n_h, :])
                    # bias column: (-m) * a_s   (overwrites the pad column)
                    nc.vector.tensor_tensor(
                        out=gth[:, :n_h, 3:4], in0=gth[:, :n_h, 0:1],
                        in1=gth[:, :n_h, 1:2], op=OP.mult,
                    )
                    # s = s * a_s + (-m * a_s)  per row on ScalarE
                    for i in range(n_h):
                        nc.scalar.activation(
                            out=s_t[:, lo + i, :], in_=s_t[:, lo + i, :],
                            func=AF.Identity,
                            bias=gth[:, i, 3:4], scale=gth[:, i, 1:2],
                        )
                    # v = v * a_v: batched on DVE; tail rows of the last half
                    # go per-row to ScalarE for engine balance
                    va = vn_act_rows if hi == r else 0
                    nd = n_h - va
                    if nd:
                        nc.vector.tensor_tensor(
                            out=v_t[:, lo : lo + nd, :],
                            in0=v_t[:, lo : lo + nd, :],
                            in1=gth[:, :nd, 2:3].to_broadcast((P, nd, VD)),
                            op=OP.mult,
                        )
                    for i in range(nd, n_h):
                        nc.scalar.mul(
                            out=v_t[:, lo + i, :], in_=v_t[:, lo + i, :],
                            mul=gth[:, i, 2:3],
                        )
                    store.dma_start(
                        out=so_r[c][:, lo:hi, :], in_=s_t[:, lo:hi, :]
                    )
                    store.dma_start(
                        out=vo_r[c][:, lo:hi, :], in_=v_t[:, lo:hi, :]
                    )

            # normalize lags stats by TWO chunks: totals_{c-1} is already
            # final for every graph touching chunk c-2 (a graph spans < 1
            # chunk of nodes), and the extra lag takes the params/gather/
            # normalize chain off the per-chunk critical path.
            totals_hist = {}
            totals_prev = None
            for c in range(nchunk):
                load_chunk(c)
                stats_ps = stats_chunk(c)
                totals_c = small.tile([w, 4], f32, tag="totals")
                if c == 0:
                    nc.vector.tensor_copy(totals_c[:], stats_ps[:])
                else:
                    nc.vector.tensor_add(totals_c[:], totals_prev[:], stats_ps[:])
                totals_prev = totals_c
                totals_hist[c] = totals_c
                if c >= 2:
                    normalize_chunk(c - 2, params_from(totals_hist.pop(c - 1)))
            if nchunk >= 2:
                normalize_chunk(nchunk - 2, params_from(totals_prev))
            normalize_chunk(nchunk - 1, params_from(totals_prev))

    nc.compile()
    return nc


def shard_plan(index, n_cores=M_CORES, cap=None, w=None):
    """Split nodes at graph boundaries into n_cores balanced shards.

    Returns (pos, cut_graphs): pos[k] is the node start of shard k,
    cut_graphs[k] the first graph id of shard k.
    """
    bounds = np.searchsorted(index, np.arange(NUM_GRAPHS + 1))
    gper = NUM_GRAPHS // n_cores
    cut_graphs = [k * gper for k in range(n_cores + 1)]
    pos = [int(bounds[g]) for g in cut_graphs]
    if cap is not None:
        assert max(np.diff(pos)) <= cap, f"shard too big: {np.diff(pos)}"
    if w is not None:
        assert max(np.diff(cut_graphs)) <= w, "too many graphs in a shard"
    return pos, cut_graphs


_NC_CACHE = {}


def _get_nc():
    if "nc" not in _NC_CACHE:
        _NC_CACHE["nc"] = build_kernel()
    return _NC_CACHE["nc"]


def kernel(s, v, index, weight_s, bias_s, _trace=False):
    s = np.asarray(s, dtype=np.float32)
    v = np.asarray(v, dtype=np.float32)
    index = np.asarray(index)
    weight_s = np.asarray(weight_s, dtype=np.float32)
    bias_s = np.asarray(bias_s, dtype=np.float32)

    n = s.shape[0]
    ch = R_FULL * P
    cap = NCHUNK_FULL * ch
    pos, cut_graphs = shard_plan(index, M_CORES, cap, W_FULL)

    in_maps = []
    for k in range(M_CORES):
        lo, hi = pos[k], pos[k + 1]
        nk = hi - lo
        s_k = np.zeros((cap, SD), np.float32)
        s_k[:nk] = s[lo:hi]
        v_k = np.zeros((cap, VD), np.float32)
        v_k[:nk] = v[lo:hi].reshape(nk, VD)
        i_k = np.full((cap,), PAD_IDX, np.float32)
        i_k[:nk] = (index[lo:hi] - cut_graphs[k]).astype(np.float32)
        in_maps.append({"s_in": s_k, "v_in": v_k, "idx_in": i_k})

    nc = _get_nc()
    res = run_bass_kernel_spmd(
        nc, in_maps, core_ids=list(range(M_CORES)), trace=_trace
    )

    sout = np.empty((n, SD), np.float32)
    vout = np.empty((n, 3, SD), np.float32)
    for k in range(M_CORES):
        lo, hi = pos[k], pos[k + 1]
        nk = hi - lo
        sout[lo:hi] = res.results[k]["s_out"][:nk]
        vout[lo:hi] = res.results[k]["v_out"][:nk].reshape(nk, 3, SD)

    if not (np.all(weight_s == 1.0) and np.all(bias_s == 0.0)):
        sout = sout * weight_s[None, :] + bias_s[None, :]

    if _trace:
        return (sout, vout), res
    return sout, vout


# revision 29
# speedup vs baseline: 1.0392x; 1.0392x over previous
"""EquivLayerNorm (segment_reduce) Trainium2 kernel.

Problem: per-graph layer-norm statistics over sorted contiguous segments.
  s [N, 128], v [N, 3, 128], index [N] sorted in [0, 512).
  counts  = segsum(1)
  smean   = segsum(mean_d(s)) / cnt
  var     = segsum(mean_d((s - smean)^2)) / cnt   == E[mean_d(s^2)] - smean^2
  sout    = (s - smean) / max(var, eps) * weight_s + bias_s
  vmean   = segsum(mean_d(sum_c v^2)) / cnt
  vout    = v / max(vmean, eps)

Strategy:
  * Host splits nodes at graph boundaries into 8 balanced shards (whole
    graphs per shard -> per-shard stats need no cross-core reduction).
  * Each core streams its nodes once in chunks of 2048. Per chunk it
    computes per-node partial sums (ScalarE activation accum), segment-
    reduces them into per-graph slots with a one-hot matmul, and keeps a
    running total. Because a graph spans at most 2 chunks (max graph 452
    nodes < 2048), chunk c can be normalized as soon as chunk c+1's stats
    are accumulated; per-graph params are gathered back to nodes with a
    transposed one-hot matmul. Single read of s/v from HBM, single write.
"""

import numpy as np

import concourse.bacc as bacc
import concourse.bass as bass  # noqa: F401 (AP helpers)
import concourse.tile as tile
from concourse import mybir
from concourse.bass_utils import run_bass_kernel_spmd

P = 128          # partitions
SD = 128         # s feature dim
VD = 384         # flattened v dim (3*128)
NUM_GRAPHS = 512
EPS = 1e-6
M_CORES = 8

# per-core streaming geometry (full problem)
R_FULL = 16            # 128-node rows per chunk
NCHUNK_FULL = 13       # chunks per core -> capacity 26624 >= max shard 25194
W_FULL = 64            # local graph-slot window per core (64 graphs per shard)

PAD_IDX = 200          # local index given to padding nodes; outside [0, W)
VN_ACT_ROWS = 4        # v-norm rows offloaded from DVE to ScalarE per chunk
VSQ_DVE_ROWS = 8       # v^2 rows done as batched ACT square + batched DVE reduce


def build_kernel(r=R_FULL, nchunk=NCHUNK_FULL, w=W_FULL, use_f32r=True,
                 store_eng='gpsimd'):
    """Builds the Bass kernel. Returns the traced nc."""
    ch = r * P
    cap = nchunk * ch
    f32 = mybir.dt.float32
    f32r = mybir.dt.float32r if use_f32r else mybir.dt.float32
    AF = mybir.ActivationFunctionType
    OP = mybir.AluOpType

    vsq_dve_rows = min(VSQ_DVE_ROWS, r)
    vn_act_rows = min(VN_ACT_ROWS, max(r - 1, 0))

    nc = bacc.Bacc("TRN2", target_bir_lowering=False)
    s_in = nc.dram_tensor("s_in", [cap, SD], f32, kind="ExternalInput").ap()
    v_in = nc.dram_tensor("v_in", [cap, VD], f32, kind="ExternalInput").ap()
    idx_in = nc.dram_tensor("idx_in", [cap], mybir.dt.uint8, kind="ExternalInput").ap()
    s_out = nc.dram_tensor("s_out", [cap, SD], f32, kind="ExternalOutput").ap()
    v_out = nc.dram_tensor("v_out", [cap, VD], f32, kind="ExternalOutput").ap()

    s_r = s_in.rearrange("(c r p) d -> c p r d", p=P, r=r)
    v_r = v_in.rearrange("(c r p) d -> c p r d", p=P, r=r)
    so_r = s_out.rearrange("(c r p) d -> c p r d", p=P, r=r)
    vo_r = v_out.rearrange("(c r p) d -> c p r d", p=P, r=r)
    idx_col_dram = idx_in.rearrange("(t p) -> p t", p=P)  # [P, nchunk*r]

    with tile.TileContext(nc) as tc, nc.allow_low_precision(
        reason="fp32r matmul operands hold pre-rounded partials"
    ):
        with (
            tc.tile_pool(name="consts", bufs=1) as consts,
            tc.tile_pool(name="sio", bufs=4) as sio,
            tc.tile_pool(name="vio", bufs=4) as vio,
            tc.tile_pool(name="idxb", bufs=4) as idxbp,
            tc.tile_pool(name="opool", bufs=2) as opool,
            tc.tile_pool(name="otpool", bufs=1) as otpool,
            tc.tile_pool(name="small", bufs=4) as small,
            tc.tile_pool(name="scr", bufs=2) as scr,
            tc.tile_pool(name="scrs2", bufs=1) as scrs2,
            tc.tile_pool(name="scrvb", bufs=1) as scrvb,
            tc.tile_pool(name="gath", bufs=3) as gathp,
            tc.tile_pool(name="ps_stats", bufs=2, space="PSUM") as ps_stats,
            tc.tile_pool(name="ps_gath", bufs=3, space="PSUM") as ps_gath,
        ):
            # constants
            iota_free_i = consts.tile([P, w], mybir.dt.int32)
            nc.gpsimd.iota(iota_free_i[:], pattern=[[1, w]], channel_multiplier=0)
            iota_free = consts.tile([P, w], mybir.dt.uint8)
            nc.vector.tensor_copy(iota_free[:], iota_free_i[:])

            iota_pc_i = consts.tile([P, 1], mybir.dt.int32)
            nc.gpsimd.iota(iota_pc_i[:], pattern=[[0, 1]], channel_multiplier=1)
            iota_pc = consts.tile([P, 1], mybir.dt.uint8)
            nc.vector.tensor_copy(iota_pc[:], iota_pc_i[:])

            # f32r constant of ones (is_equal(x, x) == 1.0); a plain memset
            # of an f32r tile fails the ISA check
            ones_row = consts.tile([P, r], f32r)
            nc.vector.tensor_tensor(
                out=ones_row[:], in0=iota_free[:, 0:r], in1=iota_free[:, 0:r],
                op=OP.is_equal,
            )

            idx_col = consts.tile([P, nchunk * r], mybir.dt.uint8)
            nc.sync.dma_start(out=idx_col[:], in_=idx_col_dram)

            held = {}

            # halves: batched ops are split in two so downstream engines
            # (PE matmuls, norms) start on the first half while the second
            # half is still being produced
            h2 = max(r // 2, 1)
            halves = [(0, h2), (h2, r)] if r > 1 else [(0, r)]

            def load_chunk(c):
                s_t = sio.tile([P, r, SD], f32, tag="s")
                v_t = vio.tile([P, r, VD], f32, tag="v")
                ib_t = idxbp.tile([P, ch], mybir.dt.uint8, tag="ib")
                nc.sync.dma_start(out=s_t[:], in_=s_r[c])
                nc.sync.dma_start(out=v_t[:], in_=v_r[c])
                nc.sync.dma_start(
                    out=ib_t[:],
                    in_=idx_in[c * ch : (c + 1) * ch][None, :].to_broadcast(
                        (P, ch)
                    ),
                )
                held[c] = (s_t, v_t, ib_t)

            def stats_chunk(c):
                """Segment-sum this chunk's per-node partials into [w, 4] PSUM.

                Per-row fixed instruction cost dominates at this size, so
                elementwise/reduce work runs as batched 3D-AP instructions
                (one per half-chunk). part is stat-major [P, 4, r] so each
                reduce writes a contiguous [P, r] slab.
                """
                s_t, v_t, _ = held[c]
                part = small.tile([P, 4, r], f32r, tag="part")
                nc.vector.tensor_copy(part[:, 3, :], ones_row[:])
                stats_ps = ps_stats.tile([w, 4], f32, tag="stats")
                vb = vsq_dve_rows
                for lo, hi in halves:
                    # batched per-node s sums
                    nc.vector.reduce_sum(
                        part[:, 0, lo:hi], s_t[:, lo:hi, :],
                        axis=mybir.AxisListType.X,
                    )
                    # batched s^2 on ScalarE, batched row-sums on DVE
                    scr_s2 = scrs2.tile([P, h2, SD], f32, tag="scr_s2")
                    nc.scalar.activation(
                        out=scr_s2[:, : hi - lo, :].rearrange("p r d -> p (r d)"),
                        in_=s_t[:, lo:hi, :].rearrange("p r d -> p (r d)"),
                        func=AF.Square,
                    )
                    nc.vector.reduce_sum(
                        part[:, 1, lo:hi], scr_s2[:, : hi - lo, :],
                        axis=mybir.AxisListType.X,
                    )
                    # v^2: batched (ACT square + DVE reduce) for rows < vb,
                    # per-row ScalarE free-dim accum for the rest
                    blo, bhi = min(lo, vb), min(hi, vb)
                    if bhi > blo:
                        scr_vb = scrvb.tile([P, h2, VD], f32, tag="scr_vb")
                        nc.scalar.activation(
                            out=scr_vb[:, : bhi - blo, :].rearrange(
                                "p r d -> p (r d)"
                            ),
                            in_=v_t[:, blo:bhi, :].rearrange("p r d -> p (r d)"),
                            func=AF.Square,
                        )
                        nc.vector.reduce_sum(
                            part[:, 2, blo:bhi], scr_vb[:, : bhi - blo, :],
                            axis=mybir.AxisListType.X,
                        )
                    for i in range(max(lo, vb), hi):
                        scr_v = scr.tile([P, VD], f32, tag="scr_v")
                        nc.scalar.activation(
                            out=scr_v[:], in_=v_t[:, i, :], func=AF.Square,
                            accum_out=part[:, 2, i : i + 1],
                        )
                    # batched one-hot for this half
                    o_t = opool.tile([P, h2, w], f32r, tag="o")
                    nc.vector.tensor_tensor(
                        out=o_t[:, : hi - lo, :],
                        in0=iota_free[:, None, :].to_broadcast((P, hi - lo, w)),
                        in1=idx_col[
                            :, c * r + lo : c * r + hi, None
                        ].to_broadcast((P, hi - lo, w)),
                        op=OP.is_equal,
                    )
                    for i in range(lo, hi):
                        nc.tensor.matmul(
                            stats_ps[:],
                            lhsT=o_t[:, i - lo, :], rhs=part[:, :, i],
                            start=(i == 0), stop=(i == r - 1),
                        )
                return stats_ps

            def params_from(totals):
                """totals [w, 4] sbuf -> G [w, 3] = (-smean, 1/var, 1/vmean).

                Only immediate-scalar tensor_scalar ops here: the AP-scalar
                variant (TensorScalarPtr) has too few instruction wait slots
                for Tile's sync and fails walrus codegen.
                """
                g_t = small.tile([w, 4], f32r, tag="G")
                t_sc = small.tile([w, 4], f32, tag="psc")
                rc = t_sc[:, 0:1]
                # rc = 1 / max(cnt, 1)
                nc.vector.tensor_scalar_max(out=rc, in0=totals[:, 3:4], scalar1=1.0)
                nc.vector.reciprocal(out=rc, in_=rc)
                # -m = -(S1 * rc) / SD
                nc.vector.tensor_tensor(
                    out=t_sc[:, 1:2], in0=totals[:, 0:1], in1=rc, op=OP.mult
                )
                nc.vector.tensor_scalar(
                    out=g_t[:, 0:1], in0=t_sc[:, 1:2], scalar1=-1.0 / SD,
                    scalar2=None, op0=OP.mult,
                )
                # Es2 = S2 * rc / SD
                nc.vector.tensor_tensor(
                    out=t_sc[:, 1:2], in0=totals[:, 1:2], in1=rc, op=OP.mult
                )
                nc.vector.tensor_scalar(
                    out=t_sc[:, 1:2], in0=t_sc[:, 1:2], scalar1=1.0 / SD,
                    scalar2=None, op0=OP.mult,
                )
                # var = max(Es2 - m^2, eps); a_s = 1/var   ((-m)^2 == m^2)
                nc.vector.tensor_tensor(
                    out=t_sc[:, 2:3], in0=g_t[:, 0:1], in1=g_t[:, 0:1], op=OP.mult
                )
                nc.vector.tensor_tensor(
                    out=t_sc[:, 1:2], in0=t_sc[:, 1:2], in1=t_sc[:, 2:3],
                    op=OP.subtract,
                )
                nc.vector.tensor_scalar_max(
                    out=t_sc[:, 1:2], in0=t_sc[:, 1:2], scalar1=EPS
                )
                nc.vector.reciprocal(out=g_t[:, 1:2], in_=t_sc[:, 1:2])
                # vmean = max(V * rc / SD, eps); a_v = 1/vmean
                nc.vector.tensor_tensor(
                    out=t_sc[:, 3:4], in0=totals[:, 2:3], in1=rc, op=OP.mult
                )
                nc.vector.tensor_scalar(
                    out=t_sc[:, 3:4], in0=t_sc[:, 3:4], scalar1=1.0 / SD,
                    scalar2=None, op0=OP.mult,
                )
                nc.vector.tensor_scalar_max(
                    out=t_sc[:, 3:4], in0=t_sc[:, 3:4], scalar1=EPS
                )
                nc.vector.reciprocal(out=g_t[:, 2:3], in_=t_sc[:, 3:4])
                # pad column so the fp32r gather matmul has an even free dim
                nc.vector.tensor_copy(g_t[:, 3:4], g_t[:, 2:3])
                return g_t

            def normalize_chunk(c, g_t):
                s_t, v_t, ib_t = held.pop(c)
                store = getattr(nc, store_eng)
                for lo, hi in halves:
                    n_h = hi - lo
                    # batched transposed one-hot for this half [P, n_h, 128]
                    ot_t = otpool.tile([P, h2, P], f32r, tag="ot")
                    nc.vector.tensor_tensor(
                        out=ot_t[:, :n_h, :],
                        in0=ib_t[:, lo * P : hi * P].rearrange(
                            "p (r n) -> p r n", r=n_h
                        ),
                        in1=iota_pc[:, :, None].to_broadcast((P, n_h, P)),
                        op=OP.is_equal,
                    )
                    # the half's row-gathers land in one PSUM tile -> one copy
                    g_ps = ps_gath.tile([P, h2, 4], f32, tag="gps")
                    for i in range(n_h):
                        nc.tensor.matmul(
                            g_ps[:, i, :],
                            lhsT=ot_t[:w, i, :], rhs=g_t[:w, :],
                            start=True, stop=True,
                        )
                    gth = gathp.tile([P, h2, 4], f32, tag="gth")
                    nc.vector.tensor_copy(gth[:, :n_h, :], g_ps[:, :n_h, :])
                    # bias column: (-m) * a_s   (overwrites the pad column)
                    nc.vector.tensor_tensor(
                        out=gth[:, :n_h, 3:4], in0=gth[:, :n_h, 0:1],
                        in1=gth[:, :n_h, 1:2], op=OP.mult,
                    )
                    # s = s * a_s + (-m * a_s)  per row on ScalarE
                    for i in range(n_h):
                        nc.scalar.activation(
                            out=s_t[:, lo + i, :], in_=s_t[:, lo + i, :],
                            func=AF.Identity,
                            bias=gth[:, i, 3:4], scale=gth[:, i, 1:2],
                        )
                    # v = v * a_v: batched on DVE; tail rows of the last half
                    # go per-row to ScalarE for engine balance
                    va = vn_act_rows if hi == r else 0
                    nd = n_h - va
                    if nd:
                        nc.vector.tensor_tensor(
                            out=v_t[:, lo : lo + nd, :],
                            in0=v_t[:, lo : lo + nd, :],
                            in1=gth[:, :nd, 2:3].to_broadcast((P, nd, VD)),
                            op=OP.mult,
                        )
                    for i in range(nd, n_h):
                        nc.scalar.mul(
                            out=v_t[:, lo + i, :], in_=v_t[:, lo + i, :],
                            mul=gth[:, i, 2:3],
                        )
                    store.dma_start(
                        out=so_r[c][:, lo:hi, :], in_=s_t[:, lo:hi, :]
                    )
                    store.dma_start(
                        out=vo_r[c][:, lo:hi, :], in_=v_t[:, lo:hi, :]
                    )

            # normalize lags stats by TWO chunks: totals_{c-1} is already
            # final for every graph touching chunk c-2 (a graph spans < 1
            # chunk of nodes), and the extra lag takes the params/gather/
            # normalize chain off the per-chunk critical path.
            totals_hist = {}
            totals_prev = None
            for c in range(nchunk):
                load_chunk(c)
                stats_ps = stats_chunk(c)
                totals_c = small.tile([w, 4], f32, tag="totals")
                if c == 0:
                    nc.vector.tensor_copy(totals_c[:], stats_ps[:])
                else:
                    nc.vector.tensor_add(totals_c[:], totals_prev[:], stats_ps[:])
                totals_prev = totals_c
                totals_hist[c] = totals_c
                if c >= 2:
                    normalize_chunk(c - 2, params_from(totals_hist.pop(c - 1)))
            if nchunk >= 2:
                normalize_chunk(nchunk - 2, params_from(totals_prev))
            normalize_chunk(nchunk - 1, params_from(totals_prev))

    nc.compile()
    return nc


def shard_plan(index, n_cores=M_CORES, cap=None, w=None):
    """Split nodes at graph boundaries into n_cores balanced shards.

    Returns (pos, cut_graphs): pos[k] is the node start of shard k,
    cut_graphs[k] the first graph id of shard k.
    """
    bounds = np.searchsorted(index, np.arange(NUM_GRAPHS + 1))
    gper = NUM_GRAPHS // n_cores
    cut_graphs = [k * gper for k in range(n_cores + 1)]
    pos = [int(bounds[g]) for g in cut_graphs]
    if cap is not None:
        assert max(np.diff(pos)) <= cap, f"shard too big: {np.diff(pos)}"
    if w is not None:
        assert max(np.diff(cut_graphs)) <= w, "too many graphs in a shard"
    return pos, cut_graphs


_NC_CACHE = {}


def _get_nc():
    if "nc" not in _NC_CACHE:
        _NC_CACHE["nc"] = build_kernel()
    return _NC_CACHE["nc"]


def kernel(s, v, index, weight_s, bias_s, _trace=False):
    s = np.asarray(s, dtype=np.float32)
    v = np.asarray(v, dtype=np.float32)
    index = np.asarray(index)
    weight_s = np.asarray(weight_s, dtype=np.float32)
    bias_s = np.asarray(bias_s, dtype=np.float32)

    n = s.shape[0]
    ch = R_FULL * P
    cap = NCHUNK_FULL * ch
    pos, cut_graphs = shard_plan(index, M_CORES, cap, W_FULL)

    in_maps = []
    for k in range(M_CORES):
        lo, hi = pos[k], pos[k + 1]
        nk = hi - lo
        s_k = np.zeros((cap, SD), np.float32)
        s_k[:nk] = s[lo:hi]
        v_k = np.zeros((cap, VD), np.float32)
        v_k[:nk] = v[lo:hi].reshape(nk, VD)
        i_k = np.full((cap,), PAD_IDX, np.uint8)
        i_k[:nk] = (index[lo:hi] - cut_graphs[k]).astype(np.uint8)
        in_maps.append({"s_in": s_k, "v_in": v_k, "idx_in": i_k})

    nc = _get_nc()
    res = run_bass_kernel_spmd(
        nc, in_maps, core_ids=list(range(M_CORES)), trace=_trace
    )

    sout = np.empty((n, SD), np.float32)
    vout = np.empty((n, 3, SD), np.float32)
    for k in range(M_CORES):
        lo, hi = pos[k], pos[k + 1]
        nk = hi - lo
        sout[lo:hi] = res.results[k]["s_out"][:nk]
        vout[lo:hi] = res.results[k]["v_out"][:nk].reshape(nk, 3, SD)

    if not (np.all(weight_s == 1.0) and np.all(bias_s == 0.0)):
        sout = sout * weight_s[None, :] + bias_s[None, :]

    if _trace:
        return (sout, vout), res
    return sout, vout
